# revision 2
# baseline (speedup 1.0000x reference)
"""ActorCriticRNN (CNN embed + GRU scan + actor/critic heads) on 8 TRN2 cores.

Data-parallel over the env/batch axis: each core handles 64 of the 512 envs,
all weights replicated, no cross-core communication.

Per-core schedule: the CNN/FC/LN/xi pipeline is tiled over 32 image-tiles of
128 images (= 2 time steps) each; the sequential GRU scan is emitted with a
one-tile lag so its elementwise chain hides under the conv matmul stream; the
actor/critic heads stream per 256-image chunk as the hidden sequence fills in.

All matmuls run in float32r (full-rate fp32 PE path, ~1e-4 rel err).
"""
import numpy as np
import concourse.bass as bass
import concourse.tile as tile
from concourse import bacc, mybir
from concourse.bass_utils import run_bass_kernel_spmd

F32 = mybir.dt.float32
F32R = mybir.dt.float32r
AF = mybir.ActivationFunctionType
ALU = mybir.AluOpType

T, B, HH, WW, C = 64, 512, 8, 8, 26
HID, FC, ACT_DIM = 256, 256, 6
NCORES = 8
BL = B // NCORES          # 64 envs per core
NIMG = T * BL             # 4096 images per core
IT = 128                  # images per tile (= 2 time steps)
NT = NIMG // IT           # 32 tiles
STEPS_PER_TILE = IT // BL # 2
LN_EPS = 1e-6

TRACE = False  # test.py flips this for the profiled run

_uid = [0]
def _nm(base):
    _uid[0] += 1
    return f"{base}_{_uid[0]}"


def _conv_w_host(k, cin, kw, kh):
    """1D-row-Toeplitz conv weight blocks: [128(part=(xi4,ci)), kh, kin, m, 128]."""
    w = np.zeros((128, kh, 2, 2, 128), np.float32)
    half = kw // 2
    for dy in range(kh):
        for kin in range(2):
            for m in range(2):
                for xi4 in range(4):
                    xi = kin * 4 + xi4
                    for xo4 in range(4):
                        xo = m * 4 + xo4
                        dx = xi - xo + half
                        if 0 <= dx < kw:
                            w[xi4 * 32:xi4 * 32 + cin, dy, kin, m,
                              xo4 * 32:xo4 * 32 + 32] = k[dy, dx, :, :]
    return w


def _prep_shared(inputs):
    """Weight tensors shared by all cores (host layouts, partition dim first)."""
    f = lambda a: np.ascontiguousarray(a, dtype=np.float32)
    conv1_k = np.asarray(inputs["conv1_k"], np.float32)
    conv2_k = np.asarray(inputs["conv2_k"], np.float32)
    conv3_k = np.asarray(inputs["conv3_k"], np.float32)
    Wi = np.asarray(inputs["Wi"], np.float32)
    Wh = np.asarray(inputs["Wh"], np.float32)
    ln_scale = np.asarray(inputs["ln_scale"], np.float32)
    ln_bias = np.asarray(inputs["ln_bias"], np.float32)

    d = {}
    d["w1"] = _conv_w_host(conv1_k, 26, 5, 5)
    d["w2"] = _conv_w_host(conv2_k, 32, 3, 3)
    d["w3"] = _conv_w_host(conv3_k, 32, 3, 3)
    d["b1"] = f(np.tile(np.asarray(inputs["conv1_b"]), 4).reshape(128, 1))
    d["b2"] = f(np.tile(np.asarray(inputs["conv2_b"]), 4).reshape(128, 1))
    d["b3"] = f(np.tile(np.asarray(inputs["conv3_b"]), 4).reshape(128, 1))

    fcr = np.asarray(inputs["cnn_fc_k"], np.float32).reshape(8, 8, 32, 256)
    fcr = fcr.reshape(8, 2, 4, 32, 256)
    d["fcw"] = f(fcr.transpose(2, 3, 1, 0, 4).reshape(128, 2, 8, 256))
    d["fcb_row"] = f(np.asarray(inputs["cnn_fc_b"]).reshape(1, 256))
    d["ones_row"] = np.ones((1, 128), np.float32)
    d["idn"] = np.eye(128, dtype=np.float32)

    # LayerNorm scale/bias folded into the GRU input projection
    Wi2 = ln_scale[:, None] * Wi
    bi2 = np.asarray(inputs["bi"], np.float32) + ln_bias @ Wi
    d["wiw"] = f(Wi2.reshape(2, 128, 6, 128).transpose(1, 0, 2, 3))
    d["bi_col"] = f(bi2.reshape(6, 128).T)
    d["whw"] = f(Wh.reshape(2, 128, 6, 128).transpose(1, 0, 2, 3))
    d["bhn_row"] = f(np.asarray(inputs["bhn"]).reshape(1, 256))

    d["afcw"] = f(np.asarray(inputs["actor_fc_k"]).reshape(2, 128, 2, 128).transpose(1, 0, 2, 3))
    d["afb"] = f(np.asarray(inputs["actor_fc_b"]).reshape(2, 128).T)
    d["aoutw"] = f(np.asarray(inputs["actor_out_k"]).reshape(2, 128, 6).transpose(1, 0, 2))
    d["aob"] = f(np.asarray(inputs["actor_out_b"]).reshape(6, 1))
    d["cfcw"] = f(np.asarray(inputs["critic_fc_k"]).reshape(2, 128, 2, 128).transpose(1, 0, 2, 3))
    d["cfb"] = f(np.asarray(inputs["critic_fc_b"]).reshape(2, 128).T)
    d["coutw"] = f(np.asarray(inputs["critic_out_k"]).reshape(2, 128, 1).transpose(1, 0, 2))
    d["cob"] = f(np.asarray(inputs["critic_out_b"]).reshape(1, 1))
    return d


def _prep_core(inputs, shared, core):
    b0 = core * BL
    obs = np.asarray(inputs["obs"], np.float32)
    dones = np.asarray(inputs["dones"])
    hidden = np.asarray(inputs["hidden"], np.float32)

    oc = obs[:, b0:b0 + BL]                        # [64, 64, 8, 8, 26]
    ocp = np.zeros((T, BL, 8, 8, 32), np.float32)
    ocp[..., :26] = oc
    a = ocp.reshape(NT, IT, 8, 2, 4, 32)           # [g, i, r, hf, x4, c]
    x1 = np.ascontiguousarray(a.transpose(0, 4, 5, 3, 2, 1)).reshape(NT, 128, 2, 8, IT)

    keepf = 1.0 - dones[:, b0:b0 + BL].astype(np.float32)   # [64, 64]
    keep_b = np.ascontiguousarray(
        np.broadcast_to(keepf.reshape(1, NIMG), (128, NIMG)))

    hloc = hidden[b0:b0 + BL]                      # [64, 256]
    h0 = hloc.T.reshape(2, 128, BL).transpose(1, 0, 2)      # [k, kc, b]
    h0m = np.ascontiguousarray((h0 * keepf[0][None, None, :]).reshape(128, 128))

    m = {"x1": x1, "keep": keep_b, "hid0m": h0m}
    m.update(shared)
    return m


def _build():
    nc = bacc.Bacc("TRN2", target_bir_lowering=False, debug=False)

    def pr(name, shape, dt_):
        return nc.declare_dram_parameter(name, list(shape), dt_, isOutput=False)

    x1 = pr("x1", (NT, 128, 2, 8, IT), F32R)
    w1 = pr("w1", (128, 5, 2, 2, 128), F32R)
    w2 = pr("w2", (128, 3, 2, 2, 128), F32R)
    w3 = pr("w3", (128, 3, 2, 2, 128), F32R)
    b1 = pr("b1", (128, 1), F32)
    b2 = pr("b2", (128, 1), F32)
    b3 = pr("b3", (128, 1), F32)
    fcw = pr("fcw", (128, 2, 8, 256), F32R)
    fcb_row = pr("fcb_row", (1, 256), F32R)
    ones_row = pr("ones_row", (1, 128), F32R)
    idn = pr("idn", (128, 128), F32R)
    wiw = pr("wiw", (128, 2, 6, 128), F32R)
    bi_col = pr("bi_col", (128, 6), F32)
    whw = pr("whw", (128, 2, 6, 128), F32R)
    bhn_row = pr("bhn_row", (1, 256), F32R)
    afcw = pr("afcw", (128, 2, 2, 128), F32R)
    afb = pr("afb", (128, 2), F32)
    aoutw = pr("aoutw", (128, 2, 6), F32R)
    aob = pr("aob", (6, 1), F32)
    cfcw = pr("cfcw", (128, 2, 2, 128), F32R)
    cfb = pr("cfb", (128, 2), F32)
    coutw = pr("coutw", (128, 2, 1), F32R)
    cob = pr("cob", (1, 1), F32)
    keep = pr("keep", (128, NIMG), F32)
    hid0m = pr("hid0m", (128, 128), F32R)

    o_logits = nc.declare_dram_parameter("o_logits", [6, NIMG], F32, isOutput=True)
    o_value = nc.declare_dram_parameter("o_value", [1, NIMG], F32, isOutput=True)
    o_hfin = nc.declare_dram_parameter("o_hfin", [128, 128], F32, isOutput=True)

    with tile.TileContext(nc) as tc:
        with (
            tc.tile_pool(name="wp", bufs=1) as wp,
            tc.tile_pool(name="xp", bufs=2) as xp,
            tc.tile_pool(name="sp", bufs=2) as sp,
            tc.tile_pool(name="qp", bufs=1) as qp,
            tc.tile_pool(name="pp", bufs=1, space="PSUM") as pp,
        ):
            def wt(param, shape, dt_=F32R):
                t = wp.tile(list(shape), dt_, name=_nm("w_" + param.name), tag=param.name)
                nc.sync.dma_start(t[:], param.ap())
                return t

            # ---- persistent weights / constants in SBUF ----
            t_w1 = wt(w1, (128, 5 * 2 * 2 * 128))
            t_w2 = wt(w2, (128, 3 * 2 * 2 * 128))
            t_w3 = wt(w3, (128, 3 * 2 * 2 * 128))
            t_b1 = wt(b1, (128, 1), F32)
            t_b2 = wt(b2, (128, 1), F32)
            t_b3 = wt(b3, (128, 1), F32)
            t_fcw = wt(fcw, (128, 2 * 8 * 256))
            t_fcb = wt(fcb_row, (1, 256))
            t_ones = wt(ones_row, (1, 128))
            t_idn = wt(idn, (128, 128))
            t_wiw = wt(wiw, (128, 2 * 6 * 128))
            t_bic = wt(bi_col, (128, 6), F32)
            t_whw = wt(whw, (128, 2 * 6 * 128))
            t_bhn = wt(bhn_row, (1, 256))
            t_afcw = wt(afcw, (128, 2 * 2 * 128))
            t_afb = wt(afb, (128, 2), F32)
            t_aoutw = wt(aoutw, (128, 2 * 6))
            t_aob = wt(aob, (6, 1), F32)
            t_cfcw = wt(cfcw, (128, 2 * 2 * 128))
            t_cfb = wt(cfb, (128, 2), F32)
            t_coutw = wt(coutw, (128, 2 * 1))
            t_cob = wt(cob, (1, 1), F32)
            t_keep = wt(keep, (128, NIMG), F32)

            w1v = t_w1[:].rearrange("p (d k m q) -> p d k m q", d=5, k=2, m=2)
            w2v = t_w2[:].rearrange("p (d k m q) -> p d k m q", d=3, k=2, m=2)
            w3v = t_w3[:].rearrange("p (d k m q) -> p d k m q", d=3, k=2, m=2)
            fcv = t_fcw[:].rearrange("p (k y f) -> p k y f", k=2, y=8)
            wiv = t_wiw[:].rearrange("p (k m q) -> p k m q", k=2, m=6)
            whv = t_whw[:].rearrange("p (k m q) -> p k m q", k=2, m=6)
            afv = t_afcw[:].rearrange("p (k m q) -> p k m q", k=2, m=2)
            aov = t_aoutw[:].rearrange("p (k q) -> p k q", k=2)
            cfv = t_cfcw[:].rearrange("p (k m q) -> p k m q", k=2, m=2)
            cov = t_coutw[:].rearrange("p (k q) -> p k q", k=2)

            # persistent sequence buffer [128, (kc, img)] and initial hidden
            t_seq = qp.tile([128, 2 * NIMG], F32R, name="seq", tag="seq")
            h_first = sp.tile([128, 128], F32R, name="h0", tag="hm")
            nc.sync.dma_start(h_first[:], hid0m.ap())
            h_cur = [h_first]

            def conv_layer(xin_v, wv, kh, bias_t, dst_v, is_last_evac_dve):
                """xin_v: [128, 2, 8, IT] view; dst_v same; one conv layer."""
                h2 = kh // 2
                for sub in range(IT // 64):
                    for m in range(2):
                        pc = pp.tile([128, 512], F32, name=_nm("pc"), tag="conv", bufs=2)
                        pcv = pc[:].rearrange("p (r i) -> p r i", r=8)
                        taps = [(dy, kin) for dy in range(-h2, h2 + 1)
                                for kin in range(2)]
                        for i, (dy, kin) in enumerate(taps):
                            y0 = max(0, -dy)
                            y1 = 8 - max(0, dy)
                            rhs = xin_v[:, kin, y0 + dy:y1 + dy,
                                        sub * 64:(sub + 1) * 64]
                            nc.tensor.matmul(
                                pcv[:, y0:y1, :],
                                wv[:, dy + h2, kin, m, :],
                                rhs,
                                start=(i == 0), stop=(i == len(taps) - 1))
                        dest = dst_v[:, m, :, sub * 64:(sub + 1) * 64]
                        if is_last_evac_dve and m == 1:
                            nc.vector.tensor_scalar(
                                dest, pcv, bias_t[:], 0.0, ALU.add, ALU.max)
                        else:
                            nc.scalar.activation(dest, pcv, AF.Relu, bias=bias_t[:])

            def emit_scan_step(t, xi_t):
                toff = (t % STEPS_PER_TILE) * 64
                h_m = h_cur[0]
                pg = pp.tile([128, 384], F32, name=_nm("pg"), tag="g", bufs=2)
                mms = []
                for mc in range(6):
                    out = pg[:, mc * 64:(mc + 1) * 64]
                    mm0 = nc.tensor.matmul(out, whv[:, 0, mc, :], h_m[:, 0:64],
                                           start=(mc == 0), stop=False,
                                           skip_group_check=True)
                    nc.tensor.matmul(out, whv[:, 1, mc, :], h_m[:, 64:128],
                                     start=False, stop=False,
                                     skip_group_check=True)
                    mms.append(mm0)
                # xi added via identity matmul for r,z; bhn rank-1 for n
                for mc in range(4):
                    nc.tensor.matmul(pg[:, mc * 64:(mc + 1) * 64], t_idn[:],
                                     xi_t[:, mc, toff:toff + 64],
                                     start=False, stop=False,
                                     skip_group_check=True)
                for j in range(2):
                    nc.tensor.matmul(pg[:, (4 + j) * 64:(5 + j) * 64],
                                     t_bhn[0:1, j * 128:(j + 1) * 128],
                                     t_ones[0:1, 0:64],
                                     start=False, stop=(j == 1),
                                     skip_group_check=True)
                # keep the bank-clearing first matmul first
                for later in mms[1:]:
                    tile.add_dep_helper(later.ins, mms[0].ins, sync=False,
                                        reason="psum bank-clear order")

                rz = sp.tile([128, 256], F32, name=_nm("rz"), tag="rz")
                nc.scalar.activation(rz[:], pg[:, 0:256], AF.Sigmoid)
                n1 = sp.tile([128, 128], F32, name=_nm("n1"), tag="n1")
                nc.vector.tensor_tensor(n1[:], pg[:, 256:384], rz[:, 0:128], ALU.mult)
                npre = sp.tile([128, 128], F32, name=_nm("npre"), tag="npre")
                nc.vector.tensor_tensor(
                    npre[:].rearrange("p (c i) -> p c i", c=2),
                    n1[:].rearrange("p (c i) -> p c i", c=2),
                    xi_t[:, 4:6, toff:toff + 64].bitcast(F32), ALU.add)
                nt = sp.tile([128, 128], F32, name=_nm("nt"), tag="nt")
                nc.scalar.activation(nt[:], npre[:], AF.Tanh)
                dt_ = sp.tile([128, 128], F32, name=_nm("dt"), tag="dt")
                nc.vector.tensor_tensor(dt_[:], h_m[:].bitcast(F32), nt[:], ALU.subtract)
                et = sp.tile([128, 128], F32, name=_nm("et"), tag="et")
                nc.vector.tensor_tensor(et[:], rz[:, 128:256], dt_[:], ALU.mult)
                seq_sl = t_seq[:].rearrange("p (c i) -> p c i", c=2)[:, :, t * 64:(t + 1) * 64]
                nc.vector.tensor_tensor(
                    seq_sl, nt[:].rearrange("p (c i) -> p c i", c=2),
                    et[:].rearrange("p (c i) -> p c i", c=2), ALU.add)
                if t < T - 1:
                    h_nx = sp.tile([128, 128], F32R, name=_nm("hm"), tag="hm")
                    kslice = t_keep[:, (t + 1) * 64:(t + 2) * 64]
                    nc.vector.tensor_tensor(
                        h_nx[:].rearrange("p (c i) -> p c i", c=2),
                        seq_sl.bitcast(F32),
                        kslice.rearrange("p (a i) -> p a i", a=1).broadcast_to([128, 2, 64]),
                        ALU.mult)
                    h_cur[0] = h_nx
                else:
                    nc.sync.dma_start(o_hfin.ap(), seq_sl.bitcast(F32))

            def emit_heads(c):
                cs = c * 256
                a_sb = sp.tile([128, 2 * 256], F32R, name=_nm("asb"), tag="asb")
                c_sb = sp.tile([128, 2 * 256], F32R, name=_nm("csb"), tag="csb")
                seqv = t_seq[:].rearrange("p (c i) -> p c i", c=2)
                for m in range(2):
                    pa = pp.tile([128, 256], F32, name=_nm("pa"), tag="fc")
                    nc.tensor.matmul(pa[:], afv[:, 0, m, :], seqv[:, 0, cs:cs + 256],
                                     start=True, stop=False)
                    nc.tensor.matmul(pa[:], afv[:, 1, m, :], seqv[:, 1, cs:cs + 256],
                                     start=False, stop=True)
                    nc.scalar.activation(a_sb[:, m * 256:(m + 1) * 256], pa[:],
                                         AF.Relu, bias=t_afb[:, m:m + 1])
                    pc2 = pp.tile([128, 256], F32, name=_nm("pc2"), tag="fc")
                    nc.tensor.matmul(pc2[:], cfv[:, 0, m, :], seqv[:, 0, cs:cs + 256],
                                     start=True, stop=False)
                    nc.tensor.matmul(pc2[:], cfv[:, 1, m, :], seqv[:, 1, cs:cs + 256],
                                     start=False, stop=True)
                    nc.scalar.activation(c_sb[:, m * 256:(m + 1) * 256], pc2[:],
                                         AF.Relu, bias=t_cfb[:, m:m + 1])
                pl = pp.tile([6, 256], F32, name=_nm("pl"), tag="tp")
                nc.tensor.matmul(pl[:], aov[:, 0, :], a_sb[:, 0:256],
                                 start=True, stop=False)
                nc.tensor.matmul(pl[:], aov[:, 1, :], a_sb[:, 256:512],
                                 start=False, stop=True)
                lg = sp.tile([6, 256], F32, name=_nm("lg"), tag="lg")
                nc.scalar.activation(lg[:], pl[:], AF.Identity, bias=t_aob[:])
                nc.sync.dma_start(o_logits.ap()[:, cs:cs + 256], lg[:])
                pv = pp.tile([1, 256], F32, name=_nm("pv"), tag="tp")
                nc.tensor.matmul(pv[:], cov[:, 0, :], c_sb[:, 0:256],
                                 start=True, stop=False)
                nc.tensor.matmul(pv[:], cov[:, 1, :], c_sb[:, 256:512],
                                 start=False, stop=True)
                vv = sp.tile([1, 256], F32, name=_nm("vv"), tag="vv")
                nc.scalar.activation(vv[:], pv[:], AF.Identity, bias=t_cob[:])
                nc.sync.dma_start(o_value.ap()[:, cs:cs + 256], vv[:])

            xi_tiles = {}
            for g in range(NT):
                # ---- conv pipeline for tile g ----
                t_x1 = xp.tile([128, 2 * 8 * IT], F32R, name=_nm("x1t"), tag="x1")
                nc.sync.dma_start(t_x1[:], x1.ap()[g])
                x1v = t_x1[:].rearrange("p (k r i) -> p k r i", k=2, r=8)
                t_x2 = xp.tile([128, 2 * 8 * IT], F32R, name=_nm("x2t"), tag="x2")
                x2v = t_x2[:].rearrange("p (k r i) -> p k r i", k=2, r=8)
                conv_layer(x1v, w1v, 5, t_b1, x2v, True)
                t_x3 = xp.tile([128, 2 * 8 * IT], F32R, name=_nm("x3t"), tag="x3")
                x3v = t_x3[:].rearrange("p (k r i) -> p k r i", k=2, r=8)
                conv_layer(x2v, w2v, 3, t_b2, x3v, True)
                t_y3 = xp.tile([128, 2 * 8 * IT], F32R, name=_nm("y3t"), tag="y3")
                y3v = t_y3[:].rearrange("p (k r i) -> p k r i", k=2, r=8)
                conv_layer(x3v, w3v, 3, t_b3, y3v, True)

                # ---- FC -> relu -> LN stats -> standardize -> transpose ----
                pfc = pp.tile([128, 256], F32, name=_nm("pfc"), tag="fc")
                kcs = [(hf, y) for hf in range(2) for y in range(8)]
                for i, (hf, y) in enumerate(kcs):
                    nc.tensor.matmul(pfc[:], y3v[:, hf, y, :], fcv[:, hf, y, :],
                                     start=(i == 0), stop=False,
                                     skip_group_check=True)
                nc.tensor.matmul(pfc[:], t_ones[:], t_fcb[:],
                                 start=False, stop=True, skip_group_check=True)
                emb = sp.tile([128, 256], F32, name=_nm("emb"), tag="emb")
                ssum = sp.tile([128, 1], F32, name=_nm("ssum"), tag="ssum")
                nc.scalar.activation(emb[:], pfc[:], AF.Relu, accum_out=ssum[:])
                scr = sp.tile([128, 256], F32, name=_nm("scr"), tag="scr")
                sqs = sp.tile([128, 1], F32, name=_nm("sqs"), tag="sqs")
                nc.scalar.activation(scr[:], emb[:], AF.Square, accum_out=sqs[:])
                mu = sp.tile([128, 1], F32, name=_nm("mu"), tag="mu")
                nc.vector.tensor_scalar(mu[:], ssum[:], 1.0 / 256, None, ALU.mult)
                ex2 = sp.tile([128, 1], F32, name=_nm("ex2"), tag="ex2")
                nc.vector.tensor_scalar(ex2[:], sqs[:], 1.0 / 256, LN_EPS,
                                        ALU.mult, ALU.add)
                mu2 = sp.tile([128, 1], F32, name=_nm("mu2"), tag="mu2")
                nc.vector.tensor_tensor(mu2[:], mu[:], mu[:], ALU.mult)
                var = sp.tile([128, 1], F32, name=_nm("var"), tag="var")
                nc.vector.tensor_tensor(var[:], ex2[:], mu2[:], ALU.subtract)
                sd = sp.tile([128, 1], F32, name=_nm("sd"), tag="sd")
                nc.scalar.activation(sd[:], var[:], AF.Sqrt)
                istd = sp.tile([128, 1], F32, name=_nm("istd"), tag="istd")
                nc.vector.reciprocal(istd[:], sd[:])
                emb_n = sp.tile([128, 256], F32R, name=_nm("embn"), tag="embn")
                nc.vector.tensor_scalar(emb_n[:], emb[:], mu[:], istd[:],
                                        ALU.subtract, ALU.mult)
                embT = sp.tile([128, 2 * 128], F32R, name=_nm("embT"), tag="embT")
                for kc in range(2):
                    ptp = pp.tile([128, 128], F32, name=_nm("ptp"), tag="tp")
                    nc.tensor.transpose(ptp[:].bitcast(F32R),
                                        emb_n[:, kc * 128:(kc + 1) * 128], t_idn[:])
                    nc.scalar.copy(embT[:, kc * 128:(kc + 1) * 128], ptp[:])

                # ---- xi = emb_n @ Wi' + bi' ----
                xi_t = qp.tile([128, 6 * IT], F32R, name=_nm("xit"), tag="xi", bufs=3)
                xiv = xi_t[:].rearrange("p (m i) -> p m i", m=6)
                for mc in range(6):
                    pxi = pp.tile([128, 128], F32, name=_nm("pxi"), tag="xi", bufs=2)
                    nc.tensor.matmul(pxi[:], wiv[:, 0, mc, :], embT[:, 0:128],
                                     start=True, stop=False)
                    nc.tensor.matmul(pxi[:], wiv[:, 1, mc, :], embT[:, 128:256],
                                     start=False, stop=True)
                    nc.scalar.activation(xiv[:, mc, :], pxi[:], AF.Identity,
                                         bias=t_bic[:, mc:mc + 1])
                xi_tiles[g] = xiv

                # ---- lagged GRU scan + heads ----
                if g >= 1:
                    gg = g - 1
                    for s in range(STEPS_PER_TILE):
                        emit_scan_step(gg * STEPS_PER_TILE + s, xi_tiles[gg])
                    del xi_tiles[gg]
                    if g >= 2 and g % 2 == 0:
                        emit_heads((g - 2) // 2)

            # tail: last tile's scan steps + last heads chunk
            gg = NT - 1
            for s in range(STEPS_PER_TILE):
                emit_scan_step(gg * STEPS_PER_TILE + s, xi_tiles[gg])
            emit_heads(NT // 2 - 1)

    nc.compile()
    return nc


_CACHED = {}


def kernel(**inputs):
    if "nc" not in _CACHED:
        _CACHED["nc"] = _build()
    nc = _CACHED["nc"]

    shared = _prep_shared(inputs)
    in_maps = [_prep_core(inputs, shared, c) for c in range(NCORES)]
    res = run_bass_kernel_spmd(nc, in_maps, core_ids=list(range(NCORES)),
                               trace=TRACE)
    if TRACE:
        _CACHED["exec_time_ns"] = res.exec_time_ns
        _CACHED["profile_json"] = res.profile_json

    fh = np.zeros((B, HID), np.float32)
    logits = np.zeros((T, B, ACT_DIM), np.float32)
    value = np.zeros((T, B), np.float32)
    for c in range(NCORES):
        b0 = c * BL
        r = res.results[c]
        lg = r["o_logits"].reshape(6, T, BL)
        logits[:, b0:b0 + BL, :] = lg.transpose(1, 2, 0)
        value[:, b0:b0 + BL] = r["o_value"].reshape(T, BL)
        hf = r["o_hfin"].reshape(128, 2, BL)
        fh[b0:b0 + BL] = hf.transpose(2, 1, 0).reshape(BL, HID)
    return fh, logits, value
